# revision 4
# baseline (speedup 1.0000x reference)
"""Trainium2 Bass kernel for nn_AttentionBlock (b=16, c=32, 128x128 spatial,
heads=8, dim_head=64).

Sharding: sequence-parallel over the flattened spatial dim N=16384 across 8
NeuronCores (2048 positions per core). Projections are per-position so they
shard exactly; the QK^T reduction over N is computed as per-core partials
followed by a 256KB AllReduce of sim=[16,64,64]; softmax is replicated.

Per-core layouts (SBUF partition dim first):
  x_bf[g*2+cc]   [128=(4 batch x 32 ch), 2048=m]  bf16   (g in {0,1} groups, cc c-chunk)
  wqT/wkT/wvT[cc][128=c-chunk, 512=f]             bf16   (transposed weights)
  woT[fc]        [128=f-chunk=(2 heads x 64 j|n), 256=o]  bf16
  q/k (transient)[128=m-chunk, 512=f]             bf16   (q has 1/8 scale folded in)
  sim psum[g]    [128=(head parity,i), 4*128=(head pair, j)] f32 (diag 64x64 blocks used)
  v_sb[g][ft]    [128=f-chunk=(parity,j), 2048=m] bf16
  At[g][fc]      [128=f-chunk=(parity,j), 256=o]  bf16   (Wout^T @ attn fused)
  out psum       [128=o-chunk, 512=m]             f32    (+bout bias on copy out)
"""

import numpy as np

N_CORES = 8
B, C, HS, WS = 16, 32, 128, 128
N = HS * WS              # 16384
NS = N // N_CORES        # 2048 per-core spatial shard
H = 8                    # heads
DH = 64                  # dim_head
QD = H * C               # 256 linear in_features
INNER = H * DH           # 512
G = B // H               # 2 groups
SCALE = DH ** -0.5       # 0.125

_CACHE = {}


def _build_nc():
    import concourse.bacc as bacc
    import concourse.mybir as mybir
    import concourse.tile as tile
    from concourse import masks

    f32 = mybir.dt.float32
    bf16 = mybir.dt.bfloat16

    nc = bacc.Bacc("TRN2", target_bir_lowering=False, debug=False,
                   num_devices=N_CORES)

    x_ext = nc.dram_tensor("x", [B, C, NS], f32, kind="ExternalInput")
    wq_ext = nc.dram_tensor("Wq", [INNER, QD], f32, kind="ExternalInput")
    wkv_ext = nc.dram_tensor("Wkv", [2 * INNER, QD], f32, kind="ExternalInput")
    wo_ext = nc.dram_tensor("Wout", [QD, INNER], f32, kind="ExternalInput")
    bout_ext = nc.dram_tensor("bout", [QD], f32, kind="ExternalInput")
    out_ext = nc.dram_tensor("out", [B, C, NS], f32, kind="ExternalOutput")

    with tile.TileContext(nc) as tc:
        from contextlib import ExitStack
        with ExitStack() as ctx:
            persist = ctx.enter_context(tc.tile_pool(name="persist", bufs=1))

            ident = persist.tile([128, 128], f32, tag="ident")
            masks.make_identity(nc, ident[:])

            # ---- weight staging DMAs ----
            wq_st = persist.tile([128, 4, 256], f32, tag="wq_st")
            nc.sync.dma_start(wq_st[:], wq_ext[:].rearrange("(t p) c -> p t c", p=128))
            wkv_st = persist.tile([128, 8, 256], f32, tag="wkv_st")
            nc.sync.dma_start(wkv_st[:], wkv_ext[:].rearrange("(t p) c -> p t c", p=128))
            wo_st = persist.tile([128, 2, 512], f32, tag="wo_st")
            nc.sync.dma_start(wo_st[:], wo_ext[:].rearrange("(t p) c -> p t c", p=128))
            bout_sb = persist.tile([128, 2], f32, tag="bout_sb")
            nc.sync.dma_start(bout_sb[:], bout_ext[:].rearrange("(t p) -> p t", p=128))

            # ---- x load + f32->bf16 convert ----
            x_bf = []
            with tc.tile_pool(name="xstage", bufs=2) as xstage:
                for g in range(G):
                    for cc in range(2):
                        xs = xstage.tile([128, NS], f32, tag="xs")
                        b0 = g * 8 + cc * 4
                        nc.sync.dma_start(
                            xs[:], x_ext[b0:b0 + 4].rearrange("a b m -> (a b) m"))
                        xb = persist.tile([128, NS], bf16, tag=f"xbf{g}{cc}")
                        nc.vector.tensor_copy(xb[:], xs[:])
                        x_bf.append(xb)

            # ---- transpose weights on PE (f32 in, bf16 out via copy) ----
            wqT = [persist.tile([128, 512], bf16, tag=f"wqT{cc}", name=f"wqT{cc}") for cc in range(2)]
            wkT = [persist.tile([128, 512], bf16, tag=f"wkT{cc}", name=f"wkT{cc}") for cc in range(2)]
            wvT = [persist.tile([128, 512], bf16, tag=f"wvT{cc}", name=f"wvT{cc}") for cc in range(2)]
            woT = [persist.tile([128, 256], bf16, tag=f"woT{fc}", name=f"woT{fc}") for fc in range(4)]
            with tc.tile_pool(name="tps", bufs=4, space="PSUM") as tps:
                for cc in range(2):
                    cs = slice(cc * 128, (cc + 1) * 128)
                    for t in range(4):
                        pt = tps.tile([128, 128], f32, tag="tp")
                        nc.tensor.transpose(pt[:], wq_st[:, t, cs], ident[:])
                        nc.scalar.copy(wqT[cc][:, t * 128:(t + 1) * 128], pt[:])
                    for t in range(4):
                        pt = tps.tile([128, 128], f32, tag="tp")
                        nc.tensor.transpose(pt[:], wkv_st[:, t, cs], ident[:])
                        nc.scalar.copy(wkT[cc][:, t * 128:(t + 1) * 128], pt[:])
                    for t in range(4):
                        pt = tps.tile([128, 128], f32, tag="tp")
                        nc.tensor.transpose(pt[:], wkv_st[:, t + 4, cs], ident[:])
                        nc.scalar.copy(wvT[cc][:, t * 128:(t + 1) * 128], pt[:])
                for oc in range(2):
                    for fj in range(4):
                        pt = tps.tile([128, 128], f32, tag="tp")
                        nc.tensor.transpose(
                            pt[:], wo_st[:, oc, fj * 128:(fj + 1) * 128], ident[:])
                        nc.scalar.copy(woT[fj][:, oc * 128:(oc + 1) * 128], pt[:])

            # ---- q/k projections + sim partial accumulation ----
            MT = NS // 128  # 16 m-chunks
            with tc.tile_pool(name="simps", bufs=1, space="PSUM") as simpool, \
                 tc.tile_pool(name="qkps", bufs=2, space="PSUM") as qkps, \
                 tc.tile_pool(name="qksb", bufs=3) as qksb:
                sim_ps = [simpool.tile([128, 512], f32, tag=f"sim{g}", name=f"sim{g}")
                          for g in range(G)]
                for g in range(G):
                    for mt in range(MT):
                        ms = slice(mt * 128, (mt + 1) * 128)
                        qp = qkps.tile([128, 512], f32, tag="qp")
                        kp = qkps.tile([128, 512], f32, tag="kp")
                        for cc in range(2):
                            nc.tensor.matmul(qp[:], x_bf[g * 2 + cc][:, ms],
                                             wqT[cc][:],
                                             start=(cc == 0), stop=(cc == 1))
                        for cc in range(2):
                            nc.tensor.matmul(kp[:], x_bf[g * 2 + cc][:, ms],
                                             wkT[cc][:],
                                             start=(cc == 0), stop=(cc == 1))
                        q_t = qksb.tile([128, 512], bf16, tag="q_t")
                        k_t = qksb.tile([128, 512], bf16, tag="k_t")
                        nc.scalar.mul(q_t[:], qp[:], SCALE)
                        nc.vector.tensor_copy(k_t[:], kp[:])
                        # one accumulation group per bank: start zeroes the
                        # whole 2KB zero region, so only the first matmul
                        # into the tile starts and only the last stops
                        for hp in range(4):
                            hs = slice(hp * 128, (hp + 1) * 128)
                            nc.tensor.matmul(sim_ps[g][:, hs], q_t[:, hs],
                                             k_t[:, hs],
                                             start=(mt == 0 and hp == 0),
                                             stop=(mt == MT - 1 and hp == 3))

                # ---- extract diagonal 64x64 blocks -> simsb ----
                simsb = persist.tile([128, 8, 64], f32, tag="simsb")
                for g in range(G):
                    for hp in range(4):
                        s = g * 4 + hp
                        nc.vector.tensor_copy(
                            simsb[0:64, s, :],
                            sim_ps[g][0:64, hp * 128:hp * 128 + 64])
                        nc.vector.tensor_copy(
                            simsb[64:128, s, :],
                            sim_ps[g][64:128, hp * 128 + 64:hp * 128 + 128])

            # ---- AllReduce sim over all 8 cores ----
            with tc.tile_pool(name="ardram", bufs=1, space="DRAM") as ardram:
                ar_in = ardram.tile([128, 512], f32, name="ar_in")
                ar_out = ardram.tile([128, 512], f32, name="ar_out")
                nc.sync.dma_start(ar_in[:], simsb[:].rearrange("p s j -> p (s j)"))
                nc.gpsimd.collective_compute(
                    "AllReduce",
                    mybir.AluOpType.add,
                    replica_groups=[list(range(N_CORES))],
                    ins=[ar_in.opt()],
                    outs=[ar_out.opt()],
                )
                attn_in = persist.tile([128, 8, 64], f32, tag="attn_in")
                nc.sync.dma_start(
                    attn_in[:].rearrange("p s j -> p (s j)"), ar_out[:])

            # ---- v projection (no AR dependency; overlaps the collective) ----
            v_sb = [[persist.tile([128, NS], bf16, tag=f"v{g}{ft}", name=f"v{g}{ft}")
                     for ft in range(4)] for g in range(G)]
            with tc.tile_pool(name="vps", bufs=2, space="PSUM") as vps:
                for g in range(G):
                    for ft in range(4):
                        fs = slice(ft * 128, (ft + 1) * 128)
                        for mt4 in range(4):
                            ms = slice(mt4 * 512, (mt4 + 1) * 512)
                            vp = vps.tile([128, 512], f32, tag="vp")
                            for cc in range(2):
                                nc.tensor.matmul(vp[:], wvT[cc][:, fs],
                                                 x_bf[g * 2 + cc][:, ms],
                                                 start=(cc == 0), stop=(cc == 1))
                            if mt4 % 2 == 0:
                                nc.vector.tensor_copy(v_sb[g][ft][:, ms], vp[:])
                            else:
                                nc.scalar.copy(v_sb[g][ft][:, ms], vp[:])

            # ---- softmax over j (free dim), per (partition=(parity,i), slot) ----
            negmax = persist.tile([128, 8], f32, tag="negmax")
            nc.vector.reduce_max(negmax[:], attn_in[:], axis=mybir.AxisListType.X,
                                 negate=True)
            shifted = persist.tile([128, 8, 64], f32, tag="shifted")
            nc.vector.tensor_add(shifted[:], attn_in[:],
                                 negmax[:].broadcast_to([128, 8, 64]))
            expt = persist.tile([128, 8, 64], f32, tag="expt")
            nc.scalar.activation(expt[:], shifted[:],
                                 mybir.ActivationFunctionType.Exp)
            sums = persist.tile([128, 8], f32, tag="sums")
            nc.vector.reduce_sum(sums[:], expt[:], axis=mybir.AxisListType.X)
            rsum = persist.tile([128, 8], f32, tag="rsum")
            nc.vector.reciprocal(rsum[:], sums[:])
            attn_bf = persist.tile([128, 8, 64], bf16, tag="attn_bf")
            nc.vector.tensor_mul(attn_bf[:], expt[:],
                                 rsum[:].broadcast_to([128, 8, 64]))

            # ---- At[g][fc] = Wout^T-contracted attn: [f=(parity,j), o] ----
            At = [[persist.tile([128, 256], bf16, tag=f"At{g}{fc}", name=f"At{g}{fc}")
                   for fc in range(4)] for g in range(G)]
            with tc.tile_pool(name="aps", bufs=2, space="PSUM") as aps:
                for g in range(G):
                    for fc in range(4):
                        ap_t = aps.tile([128, 256], f32, tag="ap_t")
                        for parity in range(2):
                            ps = slice(parity * 64, (parity + 1) * 64)
                            nc.tensor.matmul(ap_t[ps, :],
                                             attn_bf[ps, g * 4 + fc, :],
                                             woT[fc][ps, :],
                                             start=True, stop=True)
                        nc.vector.tensor_copy(At[g][fc][:], ap_t[:])

            # ---- final gemm: out[o, m] = sum_f At[f, o] * v[f, m] (+bout) ----
            with tc.tile_pool(name="ops", bufs=2, space="PSUM") as ops, \
                 tc.tile_pool(name="osb", bufs=3) as osb:
                for g in range(G):
                    for ot in range(2):
                        os_ = slice(ot * 128, (ot + 1) * 128)
                        for mt4 in range(4):
                            ms = slice(mt4 * 512, (mt4 + 1) * 512)
                            op_t = ops.tile([128, 512], f32, tag="op_t")
                            for fc in range(4):
                                nc.tensor.matmul(op_t[:], At[g][fc][:, os_],
                                                 v_sb[g][fc][:, ms],
                                                 start=(fc == 0), stop=(fc == 3))
                            o_t = osb.tile([128, 512], f32, tag="o_t")
                            nc.scalar.activation(
                                o_t[:], op_t[:],
                                mybir.ActivationFunctionType.Identity,
                                bias=bout_sb[:, ot:ot + 1])
                            b0 = g * 8 + ot * 4
                            nc.sync.dma_start(
                                out_ext[b0:b0 + 4, :, ms].rearrange(
                                    "a b m -> (a b) m"),
                                o_t[:])

    nc.compile()
    return nc


def _get_nc():
    if "nc" not in _CACHE:
        _CACHE["nc"] = _build_nc()
    return _CACHE["nc"]


def make_in_maps(x, Wq, Wkv, Wout, bout):
    xf = np.ascontiguousarray(x, dtype=np.float32).reshape(B, C, N)
    Wq = np.ascontiguousarray(Wq, dtype=np.float32)
    Wkv = np.ascontiguousarray(Wkv, dtype=np.float32)
    Wout = np.ascontiguousarray(Wout, dtype=np.float32)
    bout = np.ascontiguousarray(bout, dtype=np.float32)
    return [
        {
            "x": np.ascontiguousarray(xf[:, :, i * NS:(i + 1) * NS]),
            "Wq": Wq, "Wkv": Wkv, "Wout": Wout, "bout": bout,
        }
        for i in range(N_CORES)
    ]


def gather_out(results):
    out = np.concatenate([results[i]["out"] for i in range(N_CORES)], axis=2)
    return out.reshape(B, C, HS, WS).astype(np.float32)


def run_sharded(in_maps, **kw):
    from concourse.bass_utils import run_bass_kernel_spmd
    nc = _get_nc()
    return run_bass_kernel_spmd(nc, in_maps, list(range(N_CORES)), **kw)


def kernel(x, Wq, Wkv, Wout, bout):
    in_maps = make_in_maps(x, Wq, Wkv, Wout, bout)
    res = run_sharded(in_maps)
    return gather_out(res.results)


if __name__ == "__main__":
    nc = _get_nc()
    print("built + compiled OK")
